# revision 5
# baseline (speedup 1.0000x reference)
"""Trainium2 Bass kernel for the hypergraph-GNN-with-virtual-node problem.

Sharding: instance dim (8192) row-sharded over 8 cores (1024 rows each).
Net features are handled as a 512-row net-slice per core: the partial
net_out = net_inst_adj @ hc is ReduceScattered (each core sums+keeps its
net slice), the per-slice phi MLP output is AllGathered (bf16) for the
next layer's dense drive/sink matmuls.  Per-virtual-node segment means go
through a small AllReduce.  All heavy matmuls run in bf16 with fp32 PSUM
accumulation; LayerNorm/residual state stays fp32 in SBUF.
"""

import sys

sys.path.insert(0, "/opt/trn_rl_repo")

import numpy as np
import ml_dtypes

from concourse import bacc, tile, mybir
from concourse.bass_utils import run_bass_kernel_spmd
from concourse.alu_op_type import AluOpType
from concourse.masks import make_identity

F32 = mybir.dt.float32
BF16 = mybir.dt.bfloat16
AF = mybir.ActivationFunctionType
ADD = AluOpType.add
SUB = AluOpType.subtract
MUL = AluOpType.mult
MAX = AluOpType.max

NCORES = 8
NI, NN, E, ND, NETD, L, NVN = 8192, 4096, 256, 32, 16, 3, 64
P = NI // NCORES        # 1024 instance rows per core
NS = NN // NCORES       # 512 net rows per core
NEG = 0.1

bf = ml_dtypes.bfloat16

_CACHE = {}


def _pack_w(w):
    """[din, dout] -> [128, din/128, dout] bf16 (k-tiles on free axis)."""
    w = np.asarray(w, np.float32)
    din, dout = w.shape
    if din <= 128:
        return np.ascontiguousarray(w.astype(bf))
    k = din // 128
    return np.ascontiguousarray(w.reshape(k, 128, dout).transpose(1, 0, 2).astype(bf))


def _pack_bias_pp(b):
    """per-partition bias layout: [dout] -> [128, dout/128] f32."""
    b = np.asarray(b, np.float32)
    n = b.shape[0] // 128
    return np.ascontiguousarray(b.reshape(n, 128).T.astype(np.float32))


def _bcast(b, rows=128):
    b = np.asarray(b, np.float32)
    return np.ascontiguousarray(np.broadcast_to(b[None, :], (rows, b.shape[0])).astype(np.float32))


def _re(ap, pat, **kw):
    return ap.rearrange(pat, **kw)


def build_program():
    nc = bacc.Bacc("TRN2", target_bir_lowering=False, debug=False, num_devices=NCORES)
    din = {}

    def dram_in(name, shape, dt):
        din[name] = nc.dram_tensor(name, list(shape), dt, kind="ExternalInput").ap()
        return din[name]

    # per-core data
    dram_in("xT", (ND, P), BF16)
    dram_in("xnT", (NETD, NS), BF16)
    dram_in("driveT", (NN, P), BF16)
    dram_in("sinkT", (NN, P), BF16)
    dram_in("niT", (P, NN), BF16)
    dram_in("oh", (NVN, P), BF16)
    dram_in("ohT", (P, NVN), BF16)
    dram_in("inv_cnt", (NVN, 1), F32)
    dram_in("vn0", (NVN, E), F32)
    # weights
    dram_in("enc1_w", (ND, 2 * E), BF16)
    dram_in("enc2_w", (128, 4, E), BF16)
    dram_in("encn1_w", (NETD, E), BF16)
    dram_in("encn2_w", (128, 2, E), BF16)
    dram_in("b_enc1", (128, 4), F32)
    dram_in("b_encn1", (128, 2), F32)
    dram_in("b_encn2", (128, 2), F32)
    dram_in("bb_enc2", (128, E), F32)
    dram_in("bb_encn2", (128, E), F32)
    for l in range(L):
        for w, kt in (("phi1", 2), ("phi2", 2), ("psi1", 2), ("psi2", 2),
                      ("mlp1", 6), ("mlp2", 6)):
            dout = 768 if w == "mlp1" else E
            dram_in(f"{w}_w{l}", (128, kt, dout), BF16)
        dram_in(f"b_phi1_{l}", (128, 2), F32)
        dram_in(f"b_psi1_{l}", (128, 2), F32)
        dram_in(f"b_psi2_{l}", (128, 2), F32)
        dram_in(f"b_mlp1_{l}", (128, 6), F32)
        dram_in(f"bb_phi2_{l}", (128, E), F32)
        dram_in(f"bb_mlp2_{l}", (128, E), F32)
        dram_in(f"gb_{l}", (128, E), F32)
        dram_in(f"bnb_{l}", (128, E), F32)
    for l in range(L - 1):
        dram_in(f"m1_w{l}", (128, 2, 2 * E), BF16)
        dram_in(f"m2_w{l}", (128, 4, E), BF16)
        dram_in(f"b_m1_{l}", (128, 4), F32)
        dram_in(f"b_m2_{l}", (128, 2), F32)

    out_hn = nc.dram_tensor("out_hn", [NS, E], F32, kind="ExternalOutput").ap()

    RG = [list(range(NCORES))]

    with tile.TileContext(nc) as tc:
        with (
            tc.tile_pool(name="pers", bufs=1) as pers,
            tc.tile_pool(name="stream", bufs=2) as stream,
            tc.tile_pool(name="wpool", bufs=1) as wpool,
            tc.tile_pool(name="nistream", bufs=2) as nistream,
            tc.tile_pool(name="tmp", bufs=2) as tmp,
            tc.tile_pool(name="evt", bufs=2) as evt,
            tc.tile_pool(name="psp", bufs=1, space="PSUM") as psp,
            tc.tile_pool(name="dram", bufs=1, space="DRAM") as dram,
        ):
            # ---------- load constants / weights ----------
            def load(name, shape, dt, pool=pers):
                t = pool.tile(list(shape), dt, tag=name)
                nc.sync.dma_start(t[:], din[name][:])
                return t

            xT = load("xT", (ND, P), BF16)
            xnT = load("xnT", (NETD, NS), BF16)
            oh = load("oh", (NVN, P), BF16)
            ohT_s = pers.tile([128, P // 128, NVN], BF16, tag="ohT")
            nc.sync.dma_start(ohT_s[:], _re(din["ohT"], "(k p) v -> p k v", p=128))
            inv_cnt = load("inv_cnt", (NVN, 1), F32)
            vn_f = load("vn0", (NVN, E), F32)

            W = {}
            for name in ("enc1_w", "enc2_w", "encn1_w", "encn2_w"):
                W[name] = load(name, din[name].shape, BF16)
            for name in ("b_enc1", "b_encn1", "b_encn2", "bb_enc2", "bb_encn2"):
                W[name] = load(name, din[name].shape, F32)

            def layer_weights(l, last):
                WL = {}
                for base in ("phi1_w", "phi2_w", "psi1_w", "psi2_w",
                             "mlp1_w", "mlp2_w"):
                    t = wpool.tile(list(din[f"{base}{l}"].shape), BF16, tag=base)
                    nc.sync.dma_start(t[:], din[f"{base}{l}"][:])
                    WL[base] = t
                names = ["b_phi1", "b_psi1", "b_psi2", "b_mlp1",
                         "bb_phi2", "bb_mlp2", "gb", "bnb"]
                if not last:
                    names += ["b_m1", "b_m2"]
                for base in names:
                    t = wpool.tile(list(din[f"{base}_{l}"].shape), F32, tag=base)
                    nc.sync.dma_start(t[:], din[f"{base}_{l}"][:])
                    WL[base] = t
                if not last:
                    for base in ("m1_w", "m2_w"):
                        t = wpool.tile(list(din[f"{base}{l}"].shape), BF16, tag=base)
                        nc.sync.dma_start(t[:], din[f"{base}{l}"][:])
                        WL[base] = t
                return WL

            ident_f = pers.tile([128, 128], F32, tag="ident_f")
            make_identity(nc, ident_f[:])
            ident_b = pers.tile([128, 128], BF16, tag="ident_b")
            make_identity(nc, ident_b[:])
            eps = pers.tile([128, 1], F32, tag="eps")
            nc.vector.memset(eps[:], 1e-5)

            # persistent state
            h_im = pers.tile([128, 8, E], F32, tag="h_im")        # h  [1024,256]
            hp_bf = pers.tile([128, 8, E], BF16, tag="hp_bf")     # h' bf16
            hc_f = pers.tile([128, 8, E], F32, tag="hc_f")
            hc_bf = pers.tile([128, 8, E], BF16, tag="hc_bf")
            hn_sl = pers.tile([128, 4, E], F32, tag="hn_sl")      # hn slice [512,256]
            hnT_sl = pers.tile([128, 2, NS], BF16, tag="hnT_sl")  # hn^T [256,512]
            vn_bf = pers.tile([NVN, E], BF16, tag="vn_bf")
            concat = pers.tile([128, 6, P], BF16, tag="concat")   # [768,1024]
            nag = pers.tile([128, 32, E], BF16, tag="nag")        # net_agg [4096,256]


            import itertools
            _psc = itertools.count()

            def psA(shape, tag="A"):
                return psp.tile(list(shape), F32, tag=tag, name=f"ps_{tag}_{next(_psc)}")

            nc.vector.tensor_copy(vn_bf[:], vn_f[:])

            def act_evict(ps, n_m, dst, bias, func=AF.Prelu, dst_off=0, n=None):
                """psum [128, n_m, n] --ACT--> dst[:, dst_off+m, :] with bias/prelu."""
                for m in range(n_m):
                    o = dst[:, dst_off + m, :] if n is None else dst[:, dst_off + m, n[0]:n[1]]
                    nc.scalar.activation(o, ps[:, m, :], func,
                                         bias=bias[:, m:m + 1], alpha=NEG)

            def leaky_dve(out, in0):
                nc.vector.scalar_tensor_tensor(out=out, in0=in0, scalar=NEG,
                                               in1=in0, op0=MUL, op1=MAX)

            def ln_res(src_ps_or_sb, gbt, bnbt, res, nchunks, psum_src):
                """res[:, i, :] = leaky(ln(src[:, i, :])) + res[:, i, :]."""
                for i in range(nchunks):
                    xin = src_ps_or_sb[:, i, :]
                    st6 = tmp.tile([128, 6], F32, tag="st6")
                    st2 = tmp.tile([128, 2], F32, tag="st2")
                    nc.vector.bn_stats(st6[:], xin)
                    nc.vector.bn_aggr(st2[:], st6[:])
                    std = tmp.tile([128, 1], F32, tag="std")
                    nc.scalar.activation(std[:], st2[:, 1:2], AF.Sqrt, bias=eps[:])
                    rstd = tmp.tile([128, 1], F32, tag="rstd")
                    nc.vector.reciprocal(rstd[:], std[:])
                    xh = tmp.tile([128, E], F32, tag="xh")
                    nc.vector.tensor_scalar(out=xh[:], in0=xin, scalar1=st2[:, 0:1],
                                            scalar2=rstd[:], op0=SUB, op1=MUL)
                    nc.vector.tensor_tensor(xh[:], xh[:], gbt[:], MUL)
                    nc.vector.tensor_tensor(xh[:], xh[:], bnbt[:], ADD)
                    y = tmp.tile([128, E], F32, tag="lny")
                    leaky_dve(y[:], xh[:])
                    nc.vector.tensor_tensor(res[:, i, :], y[:], res[:, i, :], ADD)

            # =======================  encoders  =======================
            # h0 = lr(lr(x @ enc1 + b) @ enc2 + b)
            enc_t = stream.tile([128, 4, P], BF16, tag="dchunk")
            for m in range(4):
                ps_t = psA([128, P], "ABCD"[m])
                for n in range(2):
                    nc.tensor.matmul(ps_t[:, 512 * n:512 * (n + 1)],
                                     W["enc1_w"][:, 128 * m:128 * (m + 1)],
                                     xT[:, 512 * n:512 * (n + 1)],
                                     start=True, stop=True)
                nc.scalar.activation(enc_t[:, m, :], ps_t[:], AF.Prelu,
                                     bias=W["b_enc1"][:, m:m + 1], alpha=NEG)
            ps_ha = psA([128, 4, E], "A")
            ps_hb = psA([128, 4, E], "B")
            for m in range(8):
                pt = ps_ha if m < 4 else ps_hb
                for k in range(4):
                    nc.tensor.matmul(pt[:, m % 4, :],
                                     enc_t[:, k, 128 * m:128 * (m + 1)],
                                     W["enc2_w"][:, k, :],
                                     start=(k == 0), stop=(k == 3))
            for i in range(8):
                pt = ps_ha if i < 4 else ps_hb
                xh = tmp.tile([128, E], F32, tag="xh")
                nc.vector.tensor_tensor(xh[:], pt[:, i % 4, :], W["bb_enc2"][:], ADD)
                leaky_dve(h_im[:, i, :], xh[:])

            # hn0 slice (net-major) + hnT (feature-major, recomputed)
            ps_u = psA([128, 2, NS], "C")
            for m in range(2):
                nc.tensor.matmul(ps_u[:, m, :], W["encn1_w"][:, 128 * m:128 * (m + 1)],
                                 xnT[:], start=True, stop=True)
            u_bf = pers.tile([128, 2, NS], BF16, tag="t1_bf")
            act_evict(ps_u, 2, u_bf, W["b_encn1"])
            ps_hn = psA([128, 4, E], "D")
            for m in range(4):
                for k in range(2):
                    nc.tensor.matmul(ps_hn[:, m, :],
                                     u_bf[:, k, 128 * m:128 * (m + 1)],
                                     W["encn2_w"][:, k, :],
                                     start=(k == 0), stop=(k == 1))
            for i in range(4):
                xh = tmp.tile([128, E], F32, tag="xh")
                nc.vector.tensor_tensor(xh[:], ps_hn[:, i, :], W["bb_encn2"][:], ADD)
                leaky_dve(hn_sl[:, i, :], xh[:])
            ps_ht = psA([128, 2, NS], "A")
            for m in range(2):
                for k in range(2):
                    nc.tensor.matmul(ps_ht[:, m, :],
                                     W["encn2_w"][:, k, 128 * m:128 * (m + 1)],
                                     u_bf[:, k, :], start=(k == 0), stop=(k == 1))
            act_evict(ps_ht, 2, hnT_sl, W["b_encn2"])

            # =======================  layers  =======================
            for l in range(L):
                last = l == L - 1
                WL = layer_weights(l, last)
                # ---- 1) h' = h + vn[batch] ----
                ps_va = psA([128, 4, E], "A")
                ps_vb = psA([128, 4, E], "B")
                for m in range(8):
                    pt = ps_va if m < 4 else ps_vb
                    nc.tensor.matmul(pt[:, m % 4, :], oh[:, 128 * m:128 * (m + 1)],
                                     vn_bf[:], start=True, stop=True)
                for i in range(8):
                    pt = ps_va if i < 4 else ps_vb
                    nc.vector.tensor_tensor(h_im[:, i, :], h_im[:, i, :],
                                            pt[:, i % 4, :], ADD)
                    nc.scalar.copy(hp_bf[:, i, :], h_im[:, i, :])

                # ---- 2) pooled partial -> AllReduce (async) ----
                if not last:
                    ps_pool = psA([NVN, E], "C")
                    for k in range(8):
                        nc.tensor.matmul(ps_pool[:], ohT_s[:, k, :], hp_bf[:, k, :],
                                         start=(k == 0), stop=(k == 7))
                    pooled_sb = tmp.tile([NVN, E], F32, tag="pooled_sb")
                    nc.vector.tensor_copy(pooled_sb[:], ps_pool[:])
                    ar_in = dram.tile([NVN, E], F32, tag=f"ar_in{l}")
                    ar_out = dram.tile([NVN, E], F32, tag=f"ar_out{l}")
                    nc.sync.dma_start(ar_in[:], pooled_sb[:])
                    nc.gpsimd.collective_compute(
                        "AllReduce", ADD, replica_groups=RG,
                        ins=[ar_in.opt()], outs=[ar_out.opt()])

                # ---- 3) transpose h' into concat chunks 0-1 ----
                for j in range(2):
                    for i in range(8):
                        ps_tr = psA([128, 128], "C" if i % 2 == 0 else "D")
                        nc.tensor.transpose(ps_tr[:], h_im[:, i, 128 * j:128 * (j + 1)],
                                            ident_f[:])
                        if i % 2 == 0:
                            nc.vector.tensor_copy(concat[:, j, 128 * i:128 * (i + 1)], ps_tr[:])
                        else:
                            nc.scalar.copy(concat[:, j, 128 * i:128 * (i + 1)], ps_tr[:])

                # ---- 4) net_agg slice: phi MLP on hn slice ----
                ps_p1 = psA([128, 2, NS], "C")
                for m in range(2):
                    for k in range(2):
                        nc.tensor.matmul(ps_p1[:, m, :],
                                         WL["phi1_w"][:, k, 128 * m:128 * (m + 1)],
                                         hnT_sl[:, k, :], start=(k == 0), stop=(k == 1))
                t1_bf = pers.tile([128, 2, NS], BF16, tag="t1_bf")
                act_evict(ps_p1, 2, t1_bf, WL["b_phi1"])
                ps_p2 = psA([128, 4, E], "D")
                for m in range(4):
                    for k in range(2):
                        nc.tensor.matmul(ps_p2[:, m, :],
                                         t1_bf[:, k, 128 * m:128 * (m + 1)],
                                         WL["phi2_w"][:, k, :],
                                         start=(k == 0), stop=(k == 1))
                nag_sl = tmp.tile([128, 4, E], BF16, tag="nag_sl")
                for m in range(4):
                    nc.vector.tensor_tensor(nag_sl[:, m, :], ps_p2[:, m, :],
                                            WL["bb_phi2"][:], ADD)
                ag_in = dram.tile([NS, E], BF16, tag=f"ag_in{l}")
                ag_out = dram.tile([NN, E], BF16, tag=f"ag_out{l}")
                nc.sync.dma_start(_re(ag_in[:], "(k p) e -> p k e", p=128), nag_sl[:])
                nc.gpsimd.collective_compute(
                    "AllGather", AluOpType.bypass, replica_groups=RG,
                    ins=[ag_in.opt()], outs=[ag_out.opt()])
                nc.sync.dma_start(nag[:], _re(ag_out[:], "(k p) e -> p k e", p=128))

                # ---- 5) drive / sink dense matmuls ----
                hsk_bf = pers.tile([128, 2, P], BF16, tag="hsk_bf")
                for si, srcname in enumerate(("driveT", "sinkT")):
                    ps_m0 = psA([128, P], "A" if si == 0 else "C")
                    ps_m1 = psA([128, P], "B" if si == 0 else "D")
                    for kc in range(8):
                        dchunk = stream.tile([128, 4, P], BF16, tag="dchunk")
                        nc.sync.dma_start(
                            dchunk[:],
                            _re(din[srcname], "(a p) n -> p a n", p=128)[:, 4 * kc:4 * kc + 4, :])
                        for a in range(4):
                            k = 4 * kc + a
                            for m, pt in enumerate((ps_m0, ps_m1)):
                                for n in range(2):
                                    nc.tensor.matmul(
                                        pt[:, 512 * n:512 * (n + 1)],
                                        nag[:, k, 128 * m:128 * (m + 1)],
                                        dchunk[:, a, 512 * n:512 * (n + 1)],
                                        start=(k == 0), stop=(k == 31))
                    if si == 0:
                        nc.vector.tensor_copy(concat[:, 2, :], ps_m0[:])
                        nc.vector.tensor_copy(concat[:, 3, :], ps_m1[:])
                    else:
                        nc.scalar.copy(hsk_bf[:, 0, :], ps_m0[:])
                        nc.scalar.copy(hsk_bf[:, 1, :], ps_m1[:])

                # ---- 6) psi MLP on sink output -> concat chunks 4-5 ----
                s1_bf = pers.tile([128, 2, P], BF16, tag="s1_bf")
                ps_sa = psA([128, P], "A")
                ps_sb = psA([128, P], "B")
                for m, pt in enumerate((ps_sa, ps_sb)):
                    for k in range(2):
                        for n in range(2):
                            nc.tensor.matmul(pt[:, 512 * n:512 * (n + 1)],
                                             WL["psi1_w"][:, k, 128 * m:128 * (m + 1)],
                                             hsk_bf[:, k, 512 * n:512 * (n + 1)],
                                             start=(k == 0), stop=(k == 1))
                    nc.scalar.activation(s1_bf[:, m, :], pt[:], AF.Prelu,
                                         bias=WL["b_psi1"][:, m:m + 1], alpha=NEG)
                ps_sc = psA([128, P], "C")
                ps_sd = psA([128, P], "D")
                for m, pt in enumerate((ps_sc, ps_sd)):
                    for k in range(2):
                        for n in range(2):
                            nc.tensor.matmul(pt[:, 512 * n:512 * (n + 1)],
                                             WL["psi2_w"][:, k, 128 * m:128 * (m + 1)],
                                             s1_bf[:, k, 512 * n:512 * (n + 1)],
                                             start=(k == 0), stop=(k == 1))
                    nc.vector.tensor_scalar_add(concat[:, 4 + m, :], pt[:],
                                                WL["b_psi2"][:, m:m + 1])

                # ---- 7) mlp1 / mlp2 ----
                c1_bf = pers.tile([128, 6, P], BF16, tag="c1_bf")
                for n in range(2):
                    pts = [psA([128, 2, 512], t) for t in "ABC"]
                    for m in range(6):
                        for k in range(6):
                            nc.tensor.matmul(pts[m // 2][:, m % 2, :],
                                             WL["mlp1_w"][:, k, 128 * m:128 * (m + 1)],
                                             concat[:, k, 512 * n:512 * (n + 1)],
                                             start=(k == 0), stop=(k == 5))
                    for m in range(6):
                        nc.scalar.activation(c1_bf[:, m, 512 * n:512 * (n + 1)],
                                             pts[m // 2][:, m % 2, :], AF.Prelu,
                                             bias=WL["b_mlp1"][:, m:m + 1], alpha=NEG)
                ps_ca = psA([128, 4, E], "A")
                ps_cb = psA([128, 4, E], "B")
                for m in range(8):
                    pt = ps_ca if m < 4 else ps_cb
                    for k in range(6):
                        nc.tensor.matmul(pt[:, m % 4, :],
                                         c1_bf[:, k, 128 * m:128 * (m + 1)],
                                         WL["mlp2_w"][:, k, :],
                                         start=(k == 0), stop=(k == 5))
                for i in range(8):
                    pt = ps_ca if i < 4 else ps_cb
                    nc.vector.tensor_tensor(hc_f[:, i, :], pt[:, i % 4, :],
                                            WL["bb_mlp2"][:], ADD)
                    nc.scalar.copy(hc_bf[:, i, :], hc_f[:, i, :])

                # ---- 8) net_out partial = niT.T @ hc  -> ReduceScatter ----
                rs_in = dram.tile([NN, E], F32, tag=f"rs_in{l}")
                rs_out = dram.tile([NS, E], F32, tag=f"rs_out{l}")
                rs_in_re = _re(rs_in[:], "(t p) e -> p t e", p=128)
                for mh in range(2):
                    grp = [psA([128, 4, E], t) for t in "ABCD"]
                    for k in range(8):
                        nt = nistream.tile([128, NN], BF16, tag="nichunk")
                        nc.sync.dma_start(
                            nt[:], _re(din["niT"], "(k p) n -> p k n", p=128)[:, k, :])
                        for mm in range(16):
                            m = 16 * mh + mm
                            nc.tensor.matmul(grp[mm // 4][:, mm % 4, :],
                                             nt[:, 128 * m:128 * (m + 1)],
                                             hc_bf[:, k, :],
                                             start=(k == 0), stop=(k == 7))
                    for g in range(4):
                        no_t = evt.tile([128, 4, E], F32, tag="no_t")
                        for j in range(4):
                            if j % 2 == 0:
                                nc.vector.tensor_copy(no_t[:, j, :], grp[g][:, j, :])
                            else:
                                nc.scalar.copy(no_t[:, j, :], grp[g][:, j, :])
                        nc.sync.dma_start(
                            rs_in_re[:, 16 * mh + 4 * g:16 * mh + 4 * g + 4, :], no_t[:])
                nc.gpsimd.collective_compute(
                    "ReduceScatter", ADD, replica_groups=RG,
                    ins=[rs_in.opt()], outs=[rs_out.opt()])
                no_sl = pers.tile([128, 4, E], F32, tag="no_sl")
                nc.sync.dma_start(no_sl[:], _re(rs_out[:], "(k p) e -> p k e", p=128))

                # ---- 9) hn update ----
                ln_res(no_sl, WL["gb"], WL["bnb"], hn_sl, 4, True)
                if last:
                    nc.sync.dma_start(_re(out_hn, "(k p) e -> p k e", p=128), hn_sl[:])
                else:
                    for kk in range(4):
                        for j in range(2):
                            ps_tr = psA([128, 128], "C" if j == 0 else "D")
                            nc.tensor.transpose(
                                ps_tr[:], hn_sl[:, kk, 128 * j:128 * (j + 1)], ident_f[:])
                            nc.vector.tensor_copy(
                                hnT_sl[:, j, 128 * kk:128 * (kk + 1)], ps_tr[:])
                    # ---- 10) h update ----
                    ln_res(hc_f, WL["gb"], WL["bnb"], h_im, 8, False)
                    # ---- 11) vn update from pooled AllReduce ----
                    pooled = tmp.tile([NVN, E], F32, tag="pooled")
                    nc.sync.dma_start(pooled[:], ar_out[:])
                    X = tmp.tile([NVN, E], F32, tag="Xvn")
                    nc.vector.scalar_tensor_tensor(out=X[:], in0=pooled[:],
                                                   scalar=inv_cnt[:, 0:1], in1=vn_f[:],
                                                   op0=MUL, op1=ADD)
                    XT = tmp.tile([128, 2, NVN], BF16, tag="XT")
                    for j in range(2):
                        ps_x = psA([128, NVN], "A" if j == 0 else "B")
                        nc.tensor.transpose(ps_x[:], X[:, 128 * j:128 * (j + 1)],
                                            ident_f[0:NVN, 0:NVN])
                        nc.vector.tensor_copy(XT[:, j, :], ps_x[:])
                    ps_y1 = psA([128, 4, NVN], "C")
                    for m in range(4):
                        for k in range(2):
                            nc.tensor.matmul(ps_y1[:, m, :],
                                             WL["m1_w"][:, k, 128 * m:128 * (m + 1)],
                                             XT[:, k, :], start=(k == 0), stop=(k == 1))
                    y1_bf = tmp.tile([128, 4, NVN], BF16, tag="y1_bf")
                    act_evict(ps_y1, 4, y1_bf, WL["b_m1"])
                    ps_y2 = psA([128, 2, NVN], "D")
                    for m in range(2):
                        for k in range(4):
                            nc.tensor.matmul(ps_y2[:, m, :],
                                             WL["m2_w"][:, k, 128 * m:128 * (m + 1)],
                                             y1_bf[:, k, :], start=(k == 0), stop=(k == 3))
                    d_bf = tmp.tile([128, 2, NVN], BF16, tag="d_bf")
                    act_evict(ps_y2, 2, d_bf, WL["b_m2"])
                    for j in range(2):
                        ps_d = psp.tile([NVN, 128], BF16, tag="A" if j == 0 else "B", name=f"ps_d{l}_{j}")
                        nc.tensor.transpose(ps_d[:], d_bf[:, j, :], ident_b[:])
                        dtmp = tmp.tile([NVN, 128], F32, tag="dtmp")
                        nc.vector.tensor_copy(dtmp[:], ps_d[:])
                        nc.vector.tensor_tensor(vn_f[:, 128 * j:128 * (j + 1)],
                                                vn_f[:, 128 * j:128 * (j + 1)],
                                                dtmp[:], ADD)
        
            import itertools
            _psc = itertools.count()

            def psA(shape, tag="A"):
                return psp.tile(list(shape), F32, tag=tag, name=f"ps_{tag}_{next(_psc)}")

            nc.vector.tensor_copy(vn_bf[:], vn_f[:])

    nc.compile()
    return nc


def _prep_inputs(x, x_net, net_inst_adj, drive, sink, batch, num_vn, params):
    x = np.asarray(x, np.float32)
    x_net = np.asarray(x_net, np.float32)
    ni = np.asarray(net_inst_adj, np.float32)
    drive = np.asarray(drive, np.float32)
    sink = np.asarray(sink, np.float32)
    batch = np.asarray(batch).astype(np.int64)

    cnt = np.bincount(batch, minlength=NVN).astype(np.float32)
    inv_cnt = (1.0 / np.maximum(cnt, 1.0)).reshape(NVN, 1).astype(np.float32)

    p = params
    shared = {
        "inv_cnt": inv_cnt,
        "vn0": np.ascontiguousarray(
            np.broadcast_to(np.asarray(p["vn_emb"], np.float32)[0], (NVN, E))).astype(np.float32),
        "enc1_w": _pack_w(p["enc1"][0]), "enc2_w": _pack_w(p["enc2"][0]),
        "encn1_w": _pack_w(p["enc_net1"][0]), "encn2_w": _pack_w(p["enc_net2"][0]),
        "b_enc1": _pack_bias_pp(p["enc1"][1]),
        "b_encn1": _pack_bias_pp(p["enc_net1"][1]),
        "b_encn2": _pack_bias_pp(p["enc_net2"][1]),
        "bb_enc2": _bcast(p["enc2"][1]),
        "bb_encn2": _bcast(p["enc_net2"][1]),
    }
    for l in range(L):
        lp = p["layers"][l]
        shared[f"phi1_w{l}"] = _pack_w(lp["phi1"][0])
        shared[f"phi2_w{l}"] = _pack_w(lp["phi2"][0])
        shared[f"psi1_w{l}"] = _pack_w(lp["psi1"][0])
        shared[f"psi2_w{l}"] = _pack_w(lp["psi2"][0])
        shared[f"mlp1_w{l}"] = _pack_w(lp["mlp1"][0])
        shared[f"mlp2_w{l}"] = _pack_w(lp["mlp2"][0])
        shared[f"b_phi1_{l}"] = _pack_bias_pp(lp["phi1"][1])
        shared[f"b_psi1_{l}"] = _pack_bias_pp(lp["psi1"][1])
        shared[f"b_psi2_{l}"] = _pack_bias_pp(lp["psi2"][1])
        shared[f"b_mlp1_{l}"] = _pack_bias_pp(lp["mlp1"][1])
        shared[f"bb_phi2_{l}"] = _bcast(lp["phi2"][1])
        shared[f"bb_mlp2_{l}"] = _bcast(lp["mlp2"][1])
        shared[f"gb_{l}"] = _bcast(lp["ln_g"])
        shared[f"bnb_{l}"] = _bcast(lp["ln_b"])
    for l in range(L - 1):
        m1, m2 = p["vn_mlp"][l]
        shared[f"m1_w{l}"] = _pack_w(m1[0])
        shared[f"m2_w{l}"] = _pack_w(m2[0])
        shared[f"b_m1_{l}"] = _pack_bias_pp(m1[1])
        shared[f"b_m2_{l}"] = _pack_bias_pp(m2[1])

    in_maps = []
    for c in range(NCORES):
        r = slice(P * c, P * (c + 1))
        s = slice(NS * c, NS * (c + 1))
        bc = batch[r]
        ohc = np.zeros((NVN, P), np.float32)
        ohc[bc, np.arange(P)] = 1.0
        m = dict(shared)
        m["xT"] = np.ascontiguousarray(x[r].T).astype(bf)
        m["xnT"] = np.ascontiguousarray(x_net[s].T).astype(bf)
        m["driveT"] = np.ascontiguousarray(drive[r].T).astype(bf)
        m["sinkT"] = np.ascontiguousarray(sink[r].T).astype(bf)
        m["niT"] = np.ascontiguousarray(ni[:, r].T).astype(bf)
        m["oh"] = ohc.astype(bf)
        m["ohT"] = np.ascontiguousarray(ohc.T).astype(bf)
        in_maps.append(m)
    return in_maps


def kernel(x, x_net, net_inst_adj, inst_net_adj_v_drive, inst_net_adj_v_sink,
           batch, num_vn, params, _trace=False, _tmpdir=None):
    if "nc" not in _CACHE:
        _CACHE["nc"] = build_program()
    nc = _CACHE["nc"]
    in_maps = _prep_inputs(x, x_net, net_inst_adj, inst_net_adj_v_drive,
                           inst_net_adj_v_sink, batch, num_vn, params)
    kw = {}
    if _trace:
        kw = dict(trace=True, tmpdir=_tmpdir)
    res = run_bass_kernel_spmd(nc, in_maps, core_ids=list(range(NCORES)), **kw)
    _CACHE["last_result"] = res
    out = np.concatenate([res.results[c]["out_hn"] for c in range(NCORES)], axis=0)
    return out.astype(np.float32)
